# revision 29
# baseline (speedup 1.0000x reference)
"""ChebyConv (K=3) GNN kernel for 8 Trainium2 NeuronCores.

out = x@(W0-W2) + L@c + bias,  c = x@W1 + (L@x)@(2*W2)

Sharding: destination rows split across 8 cores. Edges (sorted by dest row)
are grouped per core by (dest-quad of DQ rows, source-quartile) and padded
to 128-edge chunks with a layout shared by all cores (SPMD single program).
Feature tables (x and the all-gathered c) are stored fp16 with 128-wide
rows (256B) so dma_gather lands rows directly in the fp16 lhsT layout.
Each SpMM chunk: DVE builds an fp16 selection mask [128 edges, DQ] from a
PSUM-resident iota (PSUM operand forces the 1-port DVE perf mode, which
does not block GpSimd SWDGE descriptor generation) -> PE fp16 matmul
accumulates out^T[64,DQ] in PSUM. Hop-1 results are AllGathered per
window of quads (pipelined under hop-1 compute) for the hop-2 gathers.
"""

import os
import numpy as np

CHUNK = 128          # edges per mask-matmul chunk (PE contraction dim)
DQ = 256             # dest rows per quad (mask free dim)
WINQ = 7             # quads per AllGather window
MAX_CALL_CHUNKS = 32  # 4096 indices per dma_gather call (single_packet=False)
NC = 8

LAST_EXEC_NS = None


def _edge_layout(q_of_edge, quad_of_edge, r, c, v, idx_of_edge, nquad):
    """Build the shared static slot layout for one spmm."""
    ngrp = nquad * 4
    counts = np.zeros((NC, ngrp), dtype=np.int64)
    keys = []
    orders = []
    for ci in range(NC):
        key = quad_of_edge[ci] * 4 + q_of_edge[ci]
        order = np.lexsort((c[ci], key))
        keys.append(key[order])
        orders.append(order)
        counts[ci] = np.bincount(key, minlength=ngrp)
    cg = np.maximum(1, -(-counts.max(axis=0) // CHUNK))
    grp_chunk_off = np.concatenate(([0], np.cumsum(cg)))
    tot_chunks = int(grp_chunk_off[-1])
    tot_slots = tot_chunks * CHUNK
    quad_chunk_off = [int(grp_chunk_off[t * 4]) for t in range(nquad)] + [tot_chunks]
    calls = []
    for t in range(nquad):
        for q in range(4):
            g = t * 4 + q
            c0, c1 = int(grp_chunk_off[g]), int(grp_chunk_off[g + 1])
            k = c0
            while k < c1:
                n = min(MAX_CALL_CHUNKS, c1 - k)
                calls.append((t, q, k, n))
                k += n

    # per-chunk parity: group g = quad*4 + (half*2 + parity); parity selects
    # which 64-column half of the gathered 256B pair-row holds the features
    chunk_par = np.zeros(tot_chunks, dtype=np.int64)
    for g in range(ngrp):
        chunk_par[grp_chunk_off[g]:grp_chunk_off[g + 1]] = g % 4 % 2

    per_core = []
    for ci in range(NC):
        order = orders[ci]
        key = keys[ci]
        cnt = counts[ci]
        rr = np.zeros(tot_slots, dtype=np.float32)
        vv = np.zeros(tot_slots, dtype=np.float32)
        ii = np.zeros(tot_slots, dtype=np.int16)
        within = np.arange(len(key)) - np.repeat(
            np.concatenate(([0], np.cumsum(cnt)))[:-1], cnt)
        slot = grp_chunk_off[key] * CHUNK + within
        rr[slot] = (r[ci][order] & (DQ - 1)).astype(np.float32)
        vv[slot] = v[ci][order].astype(np.float32)
        ii[slot] = idx_of_edge[ci][order].astype(np.int16)
        rr_t = np.ascontiguousarray(rr.reshape(tot_chunks, CHUNK).T)
        vv_t = np.ascontiguousarray(vv.reshape(tot_chunks, CHUNK).T)
        iw = np.ascontiguousarray(ii.reshape(tot_slots // 16, 16).T)
        iw = np.tile(iw, (8, 1))
        per_core.append((rr_t, vv_t, iw))
    return per_core, dict(tot_chunks=tot_chunks, tot_slots=tot_slots,
                          quad_chunk_off=quad_chunk_off, calls=calls,
                          chunk_par=chunk_par)


def _host_prep(x, rows, cols, vals, weight, bias):
    N, F = x.shape
    assert F == 64
    assert N % NC == 0
    shard = N // NC
    nquad = -(-shard // DQ)
    assert nquad % WINQ == 0
    nwin = nquad // WINQ
    vrows = nquad * DQ
    # feature tables are pair-packed fp16: row j = nodes (2j, 2j+1), 256B.
    # edges group by (source parity, pair-half); hp = pair rows per half.
    hp1 = ((-(-N // 2) + 1) // 2 + CHUNK - 1) // CHUNK * CHUNK   # spmm1
    hp2 = NC * vrows // 4                                        # spmm2
    assert hp1 < 32768 and hp2 < 32768
    assert (NC * vrows) % 4 == 0

    rows = np.asarray(rows).astype(np.int64)
    cols = np.asarray(cols).astype(np.int64)
    vals = np.asarray(vals, dtype=np.float32)
    x = np.asarray(x, dtype=np.float32)
    weight = np.asarray(weight, dtype=np.float32)
    bias = np.asarray(bias, dtype=np.float32)

    bounds = np.searchsorted(rows, np.arange(NC + 1) * shard)
    r_, c_, v_ = [], [], []
    for ci in range(NC):
        e0, e1 = bounds[ci], bounds[ci + 1]
        r_.append(rows[e0:e1] - ci * shard)
        c_.append(cols[e0:e1])
        v_.append(vals[e0:e1])

    # spmm1: gather pair row col>>1 from x_pad; group = half*2 + parity
    q1 = [(c >> 1) // hp1 * 2 + (c & 1) for c in c_]
    i1 = [(c >> 1) - ((c >> 1) // hp1) * hp1 for c in c_]
    # spmm2: gather from c_tbl (window AllGather -> window/rank/quad layout)
    # node row of node j: r=j//shard, lr=j-r*shard, tq=lr//DQ, w=tq//WINQ,
    #   row = w*(NC*WINQ*DQ) + r*(WINQ*DQ) + (tq%WINQ)*DQ + (lr%DQ)
    q2, i2 = [], []
    for c in c_:
        rr = c // shard
        lr = c - rr * shard
        tq = lr // DQ
        w = tq // WINQ
        trow = (w * (NC * WINQ * DQ) + rr * (WINQ * DQ)
                + (tq % WINQ) * DQ + (lr % DQ))
        pair = trow >> 1
        half = pair // hp2
        q2.append(half * 2 + (trow & 1))
        i2.append(pair - half * hp2)
    quad_dest = [r // DQ for r in r_]

    lay1_cores, lay1 = _edge_layout(q1, quad_dest, r_, c_, v_, i1, nquad)
    lay2_cores, lay2 = _edge_layout(q2, quad_dest, r_, c_, v_, i2, nquad)

    x_pad = np.zeros((2 * hp1, 2 * F), dtype=np.float16)
    x_pad.reshape(-1, F)[:N] = x.astype(np.float16)
    iota = np.tile(np.arange(DQ, dtype=np.float16), (128, 1))
    w1 = np.ascontiguousarray(weight[1])
    w2s = np.ascontiguousarray(2.0 * weight[2])
    w0m2 = np.ascontiguousarray(weight[0] - weight[2])
    biasT = np.ascontiguousarray(bias.reshape(F, 1))

    core_inputs = []
    for ci in range(NC):
        rr1, vv1, iw1 = lay1_cores[ci]
        rr2, vv2, iw2 = lay2_cores[ci]
        xq = np.zeros((F, vrows), dtype=np.float32)
        lo = ci * shard
        hi = min(lo + vrows, N)
        xq[:, :hi - lo] = x[lo:hi].T
        core_inputs.append({
            "xg": x_pad, "xq": xq,
            "rr1": rr1, "vv1": vv1, "i1": iw1,
            "rr2": rr2, "vv2": vv2, "i2": iw2,
            "iota": iota, "w1": w1, "w2s": w2s, "w0m2": w0m2, "biasT": biasT,
        })

    meta = dict(N=N, F=F, shard=shard, nquad=nquad, nwin=nwin, vrows=vrows,
                hp1=hp1, hp2=hp2, lay1=lay1, lay2=lay2)
    return core_inputs, meta


def _build_program(meta):
    import concourse.bass as bass  # noqa
    import concourse.mybir as mybir
    import concourse.tile as tile
    from concourse import bacc

    F = meta["F"]
    nquad = meta["nquad"]
    nwin = meta["nwin"]
    vrows = meta["vrows"]
    hp1, hp2 = meta["hp1"], meta["hp2"]
    lay1, lay2 = meta["lay1"], meta["lay2"]
    f32, f16, i16 = mybir.dt.float32, mybir.dt.float16, mybir.dt.int16
    AOP = mybir.AluOpType
    ACTF = mybir.ActivationFunctionType

    nc = bacc.Bacc("TRN2", target_bir_lowering=False, debug=False,
                   num_devices=NC, num_swdge_queues=4)
    xg = nc.dram_tensor("xg", [2 * hp1, 2 * F], f16, kind="ExternalInput")
    xq = nc.dram_tensor("xq", [F, vrows], f32, kind="ExternalInput")
    edge_dram = {}
    for nm, lay in (("1", lay1), ("2", lay2)):
        edge_dram["rr" + nm] = nc.dram_tensor(
            "rr" + nm, [128, lay["tot_chunks"]], f32, kind="ExternalInput")
        edge_dram["vv" + nm] = nc.dram_tensor(
            "vv" + nm, [128, lay["tot_chunks"]], f32, kind="ExternalInput")
        edge_dram["i" + nm] = nc.dram_tensor(
            "i" + nm, [128, lay["tot_slots"] // 16], i16, kind="ExternalInput")
    iota = nc.dram_tensor("iota", [128, DQ], f16, kind="ExternalInput")
    w1 = nc.dram_tensor("w1", [F, F], f32, kind="ExternalInput")
    w2s = nc.dram_tensor("w2s", [F, F], f32, kind="ExternalInput")
    w0m2 = nc.dram_tensor("w0m2", [F, F], f32, kind="ExternalInput")
    biasT = nc.dram_tensor("biasT", [F, 1], f32, kind="ExternalInput")
    outT = nc.dram_tensor("outT", [F, vrows], f32, kind="ExternalOutput")
    c_shard = nc.dram_tensor("c_shard", [vrows, F], f16)
    c_tbl = nc.dram_tensor("c_tbl", [NC * vrows // 2, 2 * F], f16,
                           addr_space="Shared")

    max_qchunks = 0
    for lay in (lay1, lay2):
        qco = lay["quad_chunk_off"]
        max_qchunks = max(max_qchunks,
                          max(qco[t + 1] - qco[t] for t in range(nquad)))

    gq = [0]

    with tile.TileContext(nc) as tc:
        with tc.tile_pool(name="const", bufs=1) as constp, \
             tc.tile_pool(name="edges", bufs=6) as edgep, \
             tc.tile_pool(name="gbuf", bufs=4) as gp, \
             tc.tile_pool(name="mask", bufs=12) as mp, \
             tc.tile_pool(name="xqp", bufs=2) as xqp, \
             tc.tile_pool(name="acc", bufs=3) as accp, \
             tc.tile_pool(name="iops", bufs=1, space="PSUM") as iops, \
             tc.tile_pool(name="ps1", bufs=4, space="PSUM") as ps1, \
             tc.tile_pool(name="ps2", bufs=2, space="PSUM") as ps2:

            iota_t = constp.tile([128, DQ], f16)
            nc.sync.dma_start(out=iota_t[:], in_=iota[:])
            if os.environ.get("MASK_PSUM_IOTA", "0") == "1":
                # fp16 iota bit-packed in a fp32 PSUM tile: the PSUM operand
                # drops the mask tensor_scalar from 2-port modes to 2x_1p
                # (dedicated DVE port), so SWDGE descriptor generation on the
                # GpSimd Q7 is never locked out of the shared SBUF port pair.
                iota_ps = iops.tile([128, DQ // 2], f32)
                nc.scalar.activation(out=iota_ps[:],
                                     in_=iota_t[:].bitcast(f32),
                                     func=ACTF.Copy)
                mask_in0 = iota_ps[:].bitcast(f16)
            else:
                mask_in0 = iota_t[:]
            w1_t = constp.tile([F, F], f32, tag="w1")
            nc.sync.dma_start(out=w1_t[:], in_=w1[:])
            w2s_t = constp.tile([F, F], f32, tag="w2s")
            nc.sync.dma_start(out=w2s_t[:], in_=w2s[:])
            w0m2_t = constp.tile([F, F], f32, tag="w0m2")
            nc.sync.dma_start(out=w0m2_t[:], in_=w0m2[:])
            bias_t = constp.tile([F, 1], f32, tag="bias")
            nc.sync.dma_start(out=bias_t[:], in_=biasT[:])

            def spmm_quad(t, tbl, lay, nm, qsz, second):
                qco = lay["quad_chunk_off"]
                c0, c1 = qco[t], qco[t + 1]
                nch = c1 - c0
                rr_t = edgep.tile([128, max_qchunks], f32, tag="rr")
                nc.sync.dma_start(out=rr_t[:, :nch],
                                  in_=edge_dram["rr" + nm][:, c0:c1])
                vv_t = edgep.tile([128, max_qchunks], f32, tag="vv")
                nc.sync.dma_start(out=vv_t[:, :nch],
                                  in_=edge_dram["vv" + nm][:, c0:c1])
                ix_t = edgep.tile([128, max_qchunks * 8], i16, tag="ix")
                nc.sync.dma_start(out=ix_t[:, :nch * 8],
                                  in_=edge_dram["i" + nm][:, c0 * 8:c1 * 8])
                g16 = gp.tile([128, max_qchunks * 2 * F], f16, tag="g16")
                for (tt, q, k0, ncall) in lay["calls"]:
                    if tt != t:
                        continue
                    nidx = ncall * CHUNK
                    rel = k0 - c0
                    nc.gpsimd.dma_gather(
                        out_ap=g16[:, rel * 2 * F:(rel + ncall) * 2 * F]
                            .rearrange("p (c e) -> p c e", e=2 * F),
                        in_ap=tbl[(q >> 1) * qsz:, :],
                        idxs_ap=ix_t[:, rel * 8:rel * 8 + nidx // 16],
                        num_idxs=nidx, num_idxs_reg=nidx, elem_size=2 * F,
                        single_packet=False, queue_num=gq[0] % 4)
                    gq[0] += 1
                par = lay["chunk_par"]
                psum = ps1.tile([F, DQ], f32)
                for j in range(nch):
                    mask = mp.tile([128, DQ], f16)
                    nc.vector.tensor_scalar(
                        out=mask[:], in0=mask_in0,
                        scalar1=rr_t[:, j:j + 1], scalar2=vv_t[:, j:j + 1],
                        op0=AOP.is_equal, op1=AOP.mult)
                    lo = j * 2 * F + int(par[c0 + j]) * F
                    nc.tensor.matmul(out=psum[:],
                                     lhsT=g16[:, lo:lo + F],
                                     rhs=mask[:],
                                     start=(j == 0),
                                     stop=(j == nch - 1) and not second)
                xq_t = xqp.tile([F, DQ], f32, tag="xq")
                nc.sync.dma_start(out=xq_t[:], in_=xq[:, t * DQ:(t + 1) * DQ])
                if not second:
                    t1t = accp.tile([F, DQ], f32, tag="t1t")
                    nc.scalar.activation(out=t1t[:], in_=psum[:], func=ACTF.Copy)
                    ps = ps2.tile([128, (DQ // 128) * F], f32)
                    for k in range(DQ // 128):
                        nc.tensor.matmul(out=ps[:, k * F:(k + 1) * F],
                                         lhsT=t1t[:, k * 128:(k + 1) * 128],
                                         rhs=w2s_t[:], start=True, stop=False)
                        nc.tensor.matmul(out=ps[:, k * F:(k + 1) * F],
                                         lhsT=xq_t[:, k * 128:(k + 1) * 128],
                                         rhs=w1_t[:], start=False, stop=True)
                    c_sb = accp.tile([128, (DQ // 128) * F], f16, tag="csb")
                    nc.scalar.activation(out=c_sb[:], in_=ps[:], func=ACTF.Copy)
                    nc.sync.dma_start(
                        out=c_shard[t * DQ:(t + 1) * DQ, :]
                            .rearrange("(k p) e -> p k e", p=128),
                        in_=c_sb[:].rearrange("p (k e) -> p k e", e=F))
                else:
                    nc.tensor.matmul(out=psum[:], lhsT=w0m2_t[:], rhs=xq_t[:],
                                     start=False, stop=True)
                    o_sb = accp.tile([F, DQ], f32, tag="osb")
                    nc.scalar.activation(out=o_sb[:], in_=psum[:],
                                         func=ACTF.Identity, bias=bias_t[:])
                    nc.sync.dma_start(out=outT[:, t * DQ:(t + 1) * DQ],
                                      in_=o_sb[:])

            def emit_ag(w):
                nc.gpsimd.collective_compute(
                    "AllGather", mybir.AluOpType.bypass,
                    replica_groups=[list(range(NC))],
                    ins=[c_shard[w * WINQ * DQ:(w + 1) * WINQ * DQ, :]],
                    outs=[c_tbl[w * NC * WINQ * DQ // 2:
                                (w + 1) * NC * WINQ * DQ // 2, :]])

            for t in range(nquad):
                spmm_quad(t, xg, lay1, "1", hp1, second=False)
                if (t + 1) % WINQ == 0:
                    emit_ag((t + 1) // WINQ - 1)
            for t in range(nquad):
                spmm_quad(t, c_tbl, lay2, "2", hp2, second=True)

    nc.compile()
    return nc


def kernel(**inputs):
    global LAST_EXEC_NS
    core_inputs, meta = _host_prep(
        inputs["x"], inputs["rows"], inputs["cols"], inputs["vals"],
        inputs["weight"], inputs["bias"])
    nc = _build_program(meta)

    trace = os.environ.get("KERNEL_TRACE", "0") == "1"
    if trace:
        try:
            import sys, types  # noqa
            if "antenv.axon_hooks" not in sys.modules:
                import antenv
                from trn_agent_boot.trn_boot import _ntff_profile_via_ctypes
                mod = types.ModuleType("antenv.axon_hooks")
                hook = _ntff_profile_via_ctypes("/opt/axon/libaxon_pjrt.so")
                mod.get_axon_ntff_profile_hook = lambda: hook
                sys.modules["antenv.axon_hooks"] = mod
                antenv.axon_hooks = mod
        except Exception:
            trace = False

    from concourse.bass_utils import run_bass_kernel_spmd
    res = run_bass_kernel_spmd(nc, core_inputs, list(range(NC)), trace=trace)
    LAST_EXEC_NS = res.exec_time_ns

    N, F, shard = meta["N"], meta["F"], meta["shard"]
    out = np.empty((N, F), dtype=np.float32)
    for ci in range(NC):
        out[ci * shard:(ci + 1) * shard] = res.results[ci]["outT"][:, :shard].T
    return out


# revision 38
# speedup vs baseline: 1.1569x; 1.1569x over previous
"""ChebyConv (K=3) GNN kernel for 8 Trainium2 NeuronCores.

out = x@(W0-W2) + L@c + bias,  c = x@W1 + (L@x)@(2*W2)

Sharding: destination rows split across 8 cores. Edges (sorted by dest row)
are grouped per core by (dest-quad of DQ rows, source-quartile) and padded
to 128-edge chunks with a layout shared by all cores (SPMD single program).
Feature tables (x and the all-gathered c) are stored fp16 with 128-wide
rows (256B) so dma_gather lands rows directly in the fp16 lhsT layout.
Each SpMM chunk: DVE builds an fp16 selection mask [128 edges, DQ] from a
PSUM-resident iota (PSUM operand forces the 1-port DVE perf mode, which
does not block GpSimd SWDGE descriptor generation) -> PE fp16 matmul
accumulates out^T[64,DQ] in PSUM. Hop-1 results are AllGathered per
window of quads (pipelined under hop-1 compute) for the hop-2 gathers.
"""

import os
import numpy as np

CHUNK = 128          # edges per mask-matmul chunk (PE contraction dim)
DQ = 256             # dest rows per quad (mask free dim)
WINQ = 7             # quads per AllGather window
MAX_CALL_CHUNKS = 32  # 4096 indices per dma_gather call (single_packet=False)
NC = 8

LAST_EXEC_NS = None


def _edge_layout(q_of_edge, quad_of_edge, r, c, v, idx_of_edge, nquad):
    """Build the shared static slot layout for one spmm."""
    ngrp = nquad * 4
    counts = np.zeros((NC, ngrp), dtype=np.int64)
    keys = []
    orders = []
    for ci in range(NC):
        key = quad_of_edge[ci] * 4 + q_of_edge[ci]
        order = np.lexsort((c[ci], key))
        keys.append(key[order])
        orders.append(order)
        counts[ci] = np.bincount(key, minlength=ngrp)
    cg = np.maximum(1, -(-counts.max(axis=0) // CHUNK))
    grp_chunk_off = np.concatenate(([0], np.cumsum(cg)))
    tot_chunks = int(grp_chunk_off[-1])
    tot_slots = tot_chunks * CHUNK
    quad_chunk_off = [int(grp_chunk_off[t * 4]) for t in range(nquad)] + [tot_chunks]
    calls = []
    for t in range(nquad):
        for q in range(4):
            g = t * 4 + q
            c0, c1 = int(grp_chunk_off[g]), int(grp_chunk_off[g + 1])
            k = c0
            while k < c1:
                n = min(MAX_CALL_CHUNKS, c1 - k)
                calls.append((t, q, k, n))
                k += n

    # per-chunk parity: group g = quad*4 + (half*2 + parity); parity selects
    # which 64-column half of the gathered 256B pair-row holds the features
    chunk_par = np.zeros(tot_chunks, dtype=np.int64)
    for g in range(ngrp):
        chunk_par[grp_chunk_off[g]:grp_chunk_off[g + 1]] = g % 4 % 2

    per_core = []
    for ci in range(NC):
        order = orders[ci]
        key = keys[ci]
        cnt = counts[ci]
        rr = np.zeros(tot_slots, dtype=np.float32)
        vv = np.zeros(tot_slots, dtype=np.float32)
        ii = np.zeros(tot_slots, dtype=np.int16)
        within = np.arange(len(key)) - np.repeat(
            np.concatenate(([0], np.cumsum(cnt)))[:-1], cnt)
        slot = grp_chunk_off[key] * CHUNK + within
        rr[slot] = (r[ci][order] & (DQ - 1)).astype(np.float32)
        vv[slot] = v[ci][order].astype(np.float32)
        ii[slot] = idx_of_edge[ci][order].astype(np.int16)
        rr_t = np.ascontiguousarray(rr.reshape(tot_chunks, CHUNK).T)
        vv_t = np.ascontiguousarray(vv.reshape(tot_chunks, CHUNK).T)
        iw = np.ascontiguousarray(ii.reshape(tot_slots // 16, 16).T)
        iw = np.tile(iw, (8, 1))
        per_core.append((rr_t, vv_t, iw))
    return per_core, dict(tot_chunks=tot_chunks, tot_slots=tot_slots,
                          quad_chunk_off=quad_chunk_off, calls=calls,
                          chunk_par=chunk_par)


def _host_prep(x, rows, cols, vals, weight, bias):
    N, F = x.shape
    assert F == 64
    assert N % NC == 0
    shard = N // NC
    nquad = -(-shard // DQ)
    assert nquad % WINQ == 0
    nwin = nquad // WINQ
    vrows = nquad * DQ
    # feature tables are pair-packed fp16: row j = nodes (2j, 2j+1), 256B.
    # edges group by (source parity, pair-half); hp = pair rows per half.
    hp1 = ((-(-N // 2) + 1) // 2 + CHUNK - 1) // CHUNK * CHUNK   # spmm1
    hp2 = NC * vrows // 4                                        # spmm2
    assert hp1 < 32768 and hp2 < 32768
    assert (NC * vrows) % 4 == 0

    rows = np.asarray(rows).astype(np.int64)
    cols = np.asarray(cols).astype(np.int64)
    vals = np.asarray(vals, dtype=np.float32)
    x = np.asarray(x, dtype=np.float32)
    weight = np.asarray(weight, dtype=np.float32)
    bias = np.asarray(bias, dtype=np.float32)

    bounds = np.searchsorted(rows, np.arange(NC + 1) * shard)
    r_, c_, v_ = [], [], []
    for ci in range(NC):
        e0, e1 = bounds[ci], bounds[ci + 1]
        r_.append(rows[e0:e1] - ci * shard)
        c_.append(cols[e0:e1])
        v_.append(vals[e0:e1])

    # spmm1: gather pair row col>>1 from x_pad; group = half*2 + parity
    q1 = [(c >> 1) // hp1 * 2 + (c & 1) for c in c_]
    i1 = [(c >> 1) - ((c >> 1) // hp1) * hp1 for c in c_]
    # spmm2: gather from c_tbl (window AllGather -> window/rank/quad layout)
    # node row of node j: r=j//shard, lr=j-r*shard, tq=lr//DQ, w=tq//WINQ,
    #   row = w*(NC*WINQ*DQ) + r*(WINQ*DQ) + (tq%WINQ)*DQ + (lr%DQ)
    q2, i2 = [], []
    for c in c_:
        rr = c // shard
        lr = c - rr * shard
        tq = lr // DQ
        w = tq // WINQ
        trow = (w * (NC * WINQ * DQ) + rr * (WINQ * DQ)
                + (tq % WINQ) * DQ + (lr % DQ))
        pair = trow >> 1
        half = pair // hp2
        q2.append(half * 2 + (trow & 1))
        i2.append(pair - half * hp2)
    quad_dest = [r // DQ for r in r_]

    lay1_cores, lay1 = _edge_layout(q1, quad_dest, r_, c_, v_, i1, nquad)
    lay2_cores, lay2 = _edge_layout(q2, quad_dest, r_, c_, v_, i2, nquad)

    x_pad = np.zeros((2 * hp1, 2 * F), dtype=np.float16)
    x_pad.reshape(-1, F)[:N] = x.astype(np.float16)
    iota = np.tile(np.arange(DQ, dtype=np.float16), (128, 1))
    w1 = np.ascontiguousarray(weight[1].astype(np.float16))
    w2s = np.ascontiguousarray((2.0 * weight[2]).astype(np.float16))
    w0m2 = np.ascontiguousarray((weight[0] - weight[2]).astype(np.float16))
    biasT = np.ascontiguousarray(bias.reshape(F, 1))

    core_inputs = []
    for ci in range(NC):
        rr1, vv1, iw1 = lay1_cores[ci]
        rr2, vv2, iw2 = lay2_cores[ci]
        xq = np.zeros((F, vrows), dtype=np.float16)
        lo = ci * shard
        hi = min(lo + vrows, N)
        xq[:, :hi - lo] = x[lo:hi].T.astype(np.float16)
        core_inputs.append({
            "xg": x_pad, "xq": xq,
            "rr1": rr1, "vv1": vv1, "i1": iw1,
            "rr2": rr2, "vv2": vv2, "i2": iw2,
            "iota": iota, "w1": w1, "w2s": w2s, "w0m2": w0m2, "biasT": biasT,
        })

    meta = dict(N=N, F=F, shard=shard, nquad=nquad, nwin=nwin, vrows=vrows,
                hp1=hp1, hp2=hp2, lay1=lay1, lay2=lay2)
    return core_inputs, meta


def _build_program(meta):
    import concourse.bass as bass  # noqa
    import concourse.mybir as mybir
    import concourse.tile as tile
    from concourse import bacc

    F = meta["F"]
    nquad = meta["nquad"]
    nwin = meta["nwin"]
    vrows = meta["vrows"]
    hp1, hp2 = meta["hp1"], meta["hp2"]
    lay1, lay2 = meta["lay1"], meta["lay2"]
    f32, f16, i16 = mybir.dt.float32, mybir.dt.float16, mybir.dt.int16
    AOP = mybir.AluOpType
    ACTF = mybir.ActivationFunctionType

    nc = bacc.Bacc("TRN2", target_bir_lowering=False, debug=False,
                   num_devices=NC, num_swdge_queues=4)
    xg = nc.dram_tensor("xg", [2 * hp1, 2 * F], f16, kind="ExternalInput")
    xq = nc.dram_tensor("xq", [F, vrows], f16, kind="ExternalInput")
    edge_dram = {}
    for nm, lay in (("1", lay1), ("2", lay2)):
        edge_dram["rr" + nm] = nc.dram_tensor(
            "rr" + nm, [128, lay["tot_chunks"]], f32, kind="ExternalInput")
        edge_dram["vv" + nm] = nc.dram_tensor(
            "vv" + nm, [128, lay["tot_chunks"]], f32, kind="ExternalInput")
        edge_dram["i" + nm] = nc.dram_tensor(
            "i" + nm, [128, lay["tot_slots"] // 16], i16, kind="ExternalInput")
    iota = nc.dram_tensor("iota", [128, DQ], f16, kind="ExternalInput")
    w1 = nc.dram_tensor("w1", [F, F], f16, kind="ExternalInput")
    w2s = nc.dram_tensor("w2s", [F, F], f16, kind="ExternalInput")
    w0m2 = nc.dram_tensor("w0m2", [F, F], f16, kind="ExternalInput")
    biasT = nc.dram_tensor("biasT", [F, 1], f32, kind="ExternalInput")
    outT = nc.dram_tensor("outT", [F, vrows], f16, kind="ExternalOutput")
    c_shard = nc.dram_tensor("c_shard", [vrows, F], f16)
    c_tbl = nc.dram_tensor("c_tbl", [NC * vrows // 2, 2 * F], f16,
                           addr_space="Shared")

    max_qchunks = 0
    for lay in (lay1, lay2):
        qco = lay["quad_chunk_off"]
        max_qchunks = max(max_qchunks,
                          max(qco[t + 1] - qco[t] for t in range(nquad)))

    gq = [0]

    with tile.TileContext(nc) as tc:
        with tc.tile_pool(name="const", bufs=1) as constp, \
             tc.tile_pool(name="edges", bufs=6) as edgep, \
             tc.tile_pool(name="gbuf", bufs=4) as gp, \
             tc.tile_pool(name="mask", bufs=12) as mp, \
             tc.tile_pool(name="xqp", bufs=2) as xqp, \
             tc.tile_pool(name="acc", bufs=3) as accp, \
             tc.tile_pool(name="iops", bufs=1, space="PSUM") as iops, \
             tc.tile_pool(name="ps1", bufs=4, space="PSUM") as ps1, \
             tc.tile_pool(name="ps2", bufs=2, space="PSUM") as ps2:

            iota_t = constp.tile([128, DQ], f16)
            nc.sync.dma_start(out=iota_t[:], in_=iota[:])
            if os.environ.get("MASK_PSUM_IOTA", "0") == "1":
                # fp16 iota bit-packed in a fp32 PSUM tile: the PSUM operand
                # drops the mask tensor_scalar from 2-port modes to 2x_1p
                # (dedicated DVE port), so SWDGE descriptor generation on the
                # GpSimd Q7 is never locked out of the shared SBUF port pair.
                iota_ps = iops.tile([128, DQ // 2], f32)
                nc.scalar.activation(out=iota_ps[:],
                                     in_=iota_t[:].bitcast(f32),
                                     func=ACTF.Copy)
                mask_in0 = iota_ps[:].bitcast(f16)
            else:
                mask_in0 = iota_t[:]
            w1_t = constp.tile([F, F], f16, tag="w1")
            nc.sync.dma_start(out=w1_t[:], in_=w1[:])
            w2s_t = constp.tile([F, F], f16, tag="w2s")
            nc.sync.dma_start(out=w2s_t[:], in_=w2s[:])
            w0m2_t = constp.tile([F, F], f16, tag="w0m2")
            nc.sync.dma_start(out=w0m2_t[:], in_=w0m2[:])
            bias_t = constp.tile([F, 1], f32, tag="bias")
            nc.sync.dma_start(out=bias_t[:], in_=biasT[:])

            def spmm_quad(t, tbl, lay, nm, qsz, second):
                qco = lay["quad_chunk_off"]
                c0, c1 = qco[t], qco[t + 1]
                nch = c1 - c0
                rr_t = edgep.tile([128, max_qchunks], f32, tag="rr")
                nc.sync.dma_start(out=rr_t[:, :nch],
                                  in_=edge_dram["rr" + nm][:, c0:c1])
                vv_t = edgep.tile([128, max_qchunks], f32, tag="vv")
                nc.sync.dma_start(out=vv_t[:, :nch],
                                  in_=edge_dram["vv" + nm][:, c0:c1])
                ix_t = edgep.tile([128, max_qchunks * 8], i16, tag="ix")
                nc.sync.dma_start(out=ix_t[:, :nch * 8],
                                  in_=edge_dram["i" + nm][:, c0 * 8:c1 * 8])
                g16 = gp.tile([128, max_qchunks * 2 * F], f16, tag="g16")
                for (tt, q, k0, ncall) in lay["calls"]:
                    if tt != t:
                        continue
                    nidx = ncall * CHUNK
                    rel = k0 - c0
                    nc.gpsimd.dma_gather(
                        out_ap=g16[:, rel * 2 * F:(rel + ncall) * 2 * F]
                            .rearrange("p (c e) -> p c e", e=2 * F),
                        in_ap=tbl[(q >> 1) * qsz:, :],
                        idxs_ap=ix_t[:, rel * 8:rel * 8 + nidx // 16],
                        num_idxs=nidx, num_idxs_reg=nidx, elem_size=2 * F,
                        single_packet=os.environ.get("SP", "0") == "1",
                        queue_num=gq[0] % 4)
                    gq[0] += 1
                par = lay["chunk_par"]
                psum = ps1.tile([F, DQ], f32)
                for j in range(nch):
                    mask = mp.tile([128, DQ], f16)
                    nc.vector.tensor_scalar(
                        out=mask[:], in0=mask_in0,
                        scalar1=rr_t[:, j:j + 1], scalar2=vv_t[:, j:j + 1],
                        op0=AOP.is_equal, op1=AOP.mult)
                    lo = j * 2 * F + int(par[c0 + j]) * F
                    nc.tensor.matmul(out=psum[:],
                                     lhsT=g16[:, lo:lo + F],
                                     rhs=mask[:],
                                     start=(j == 0),
                                     stop=(j == nch - 1) and not second)
                xq_t = xqp.tile([F, DQ], f16, tag="xq")
                nc.sync.dma_start(out=xq_t[:], in_=xq[:, t * DQ:(t + 1) * DQ])
                if not second:
                    t1t = accp.tile([F, DQ], f16, tag="t1t")
                    nc.scalar.activation(out=t1t[:], in_=psum[:], func=ACTF.Copy)
                    ps = ps2.tile([128, (DQ // 128) * F], f32)
                    for k in range(DQ // 128):
                        nc.tensor.matmul(out=ps[:, k * F:(k + 1) * F],
                                         lhsT=t1t[:, k * 128:(k + 1) * 128],
                                         rhs=w2s_t[:], start=True, stop=False)
                        nc.tensor.matmul(out=ps[:, k * F:(k + 1) * F],
                                         lhsT=xq_t[:, k * 128:(k + 1) * 128],
                                         rhs=w1_t[:], start=False, stop=True)
                    c_sb = accp.tile([128, (DQ // 128) * F], f16, tag="csb")
                    nc.scalar.activation(out=c_sb[:], in_=ps[:], func=ACTF.Copy)
                    nc.sync.dma_start(
                        out=c_shard[t * DQ:(t + 1) * DQ, :]
                            .rearrange("(k p) e -> p k e", p=128),
                        in_=c_sb[:].rearrange("p (k e) -> p k e", e=F))
                else:
                    nc.tensor.matmul(out=psum[:], lhsT=w0m2_t[:], rhs=xq_t[:],
                                     start=False, stop=True)
                    o_sb = accp.tile([F, DQ], f16, tag="osb")
                    nc.scalar.activation(out=o_sb[:], in_=psum[:],
                                         func=ACTF.Identity, bias=bias_t[:])
                    nc.sync.dma_start(out=outT[:, t * DQ:(t + 1) * DQ],
                                      in_=o_sb[:])

            def emit_ag(w):
                nc.gpsimd.collective_compute(
                    "AllGather", mybir.AluOpType.bypass,
                    replica_groups=[list(range(NC))],
                    ins=[c_shard[w * WINQ * DQ:(w + 1) * WINQ * DQ, :]],
                    outs=[c_tbl[w * NC * WINQ * DQ // 2:
                                (w + 1) * NC * WINQ * DQ // 2, :]])

            for t in range(nquad):
                spmm_quad(t, xg, lay1, "1", hp1, second=False)
                if (t + 1) % WINQ == 0:
                    emit_ag((t + 1) // WINQ - 1)
            for t in range(nquad):
                spmm_quad(t, c_tbl, lay2, "2", hp2, second=True)

    nc.compile()
    return nc


def kernel(**inputs):
    global LAST_EXEC_NS
    core_inputs, meta = _host_prep(
        inputs["x"], inputs["rows"], inputs["cols"], inputs["vals"],
        inputs["weight"], inputs["bias"])
    nc = _build_program(meta)

    trace = os.environ.get("KERNEL_TRACE", "0") == "1"
    if trace:
        try:
            import sys, types  # noqa
            if "antenv.axon_hooks" not in sys.modules:
                import antenv
                from trn_agent_boot.trn_boot import _ntff_profile_via_ctypes
                mod = types.ModuleType("antenv.axon_hooks")
                hook = _ntff_profile_via_ctypes("/opt/axon/libaxon_pjrt.so")
                mod.get_axon_ntff_profile_hook = lambda: hook
                sys.modules["antenv.axon_hooks"] = mod
                antenv.axon_hooks = mod
        except Exception:
            trace = False

    from concourse.bass_utils import run_bass_kernel_spmd
    res = run_bass_kernel_spmd(nc, core_inputs, list(range(NC)), trace=trace)
    LAST_EXEC_NS = res.exec_time_ns

    N, F, shard = meta["N"], meta["F"], meta["shard"]
    out = np.empty((N, F), dtype=np.float32)
    for ci in range(NC):
        out[ci * shard:(ci + 1) * shard] = \
            res.results[ci]["outT"][:, :shard].T.astype(np.float32)
    return out


# revision 39
# speedup vs baseline: 1.1901x; 1.0287x over previous
"""ChebyConv (K=3) GNN kernel for 8 Trainium2 NeuronCores.

out = x@(W0-W2) + L@c + bias,  c = x@W1 + (L@x)@(2*W2)

Sharding: destination rows split across 8 cores. Edges (sorted by dest row)
are grouped per core by (dest-quad of DQ rows, source-quartile) and padded
to 128-edge chunks with a layout shared by all cores (SPMD single program).
Feature tables (x and the all-gathered c) are stored fp16 with 128-wide
rows (256B) so dma_gather lands rows directly in the fp16 lhsT layout.
Each SpMM chunk: DVE builds an fp16 selection mask [128 edges, DQ] from a
PSUM-resident iota (PSUM operand forces the 1-port DVE perf mode, which
does not block GpSimd SWDGE descriptor generation) -> PE fp16 matmul
accumulates out^T[64,DQ] in PSUM. Hop-1 results are AllGathered per
window of quads (pipelined under hop-1 compute) for the hop-2 gathers.
"""

import os
import numpy as np

CHUNK = 128          # edges per mask-matmul chunk (PE contraction dim)
DQ = 256             # dest rows per quad (mask free dim)
WINQ = 7             # quads per AllGather window
MAX_CALL_CHUNKS = 32  # 4096 indices per dma_gather call (single_packet=False)
NC = 8

LAST_EXEC_NS = None


def _edge_layout(q_of_edge, quad_of_edge, r, c, v, idx_of_edge, nquad):
    """Build the shared static slot layout for one spmm."""
    ngrp = nquad * 4
    counts = np.zeros((NC, ngrp), dtype=np.int64)
    keys = []
    orders = []
    for ci in range(NC):
        key = quad_of_edge[ci] * 4 + q_of_edge[ci]
        order = np.lexsort((c[ci], key))
        keys.append(key[order])
        orders.append(order)
        counts[ci] = np.bincount(key, minlength=ngrp)
    cg = np.maximum(1, -(-counts.max(axis=0) // CHUNK))
    grp_chunk_off = np.concatenate(([0], np.cumsum(cg)))
    tot_chunks = int(grp_chunk_off[-1])
    tot_slots = tot_chunks * CHUNK
    quad_chunk_off = [int(grp_chunk_off[t * 4]) for t in range(nquad)] + [tot_chunks]
    calls = []
    for t in range(nquad):
        for q in range(4):
            g = t * 4 + q
            c0, c1 = int(grp_chunk_off[g]), int(grp_chunk_off[g + 1])
            k = c0
            while k < c1:
                n = min(MAX_CALL_CHUNKS, c1 - k)
                calls.append((t, q, k, n))
                k += n

    # per-chunk parity: group g = quad*4 + (half*2 + parity); parity selects
    # which 64-column half of the gathered 256B pair-row holds the features
    chunk_par = np.zeros(tot_chunks, dtype=np.int64)
    for g in range(ngrp):
        chunk_par[grp_chunk_off[g]:grp_chunk_off[g + 1]] = g % 4 % 2

    per_core = []
    for ci in range(NC):
        order = orders[ci]
        key = keys[ci]
        cnt = counts[ci]
        rr = np.zeros(tot_slots, dtype=np.float32)
        vv = np.zeros(tot_slots, dtype=np.float32)
        ii = np.zeros(tot_slots, dtype=np.int16)
        within = np.arange(len(key)) - np.repeat(
            np.concatenate(([0], np.cumsum(cnt)))[:-1], cnt)
        slot = grp_chunk_off[key] * CHUNK + within
        rr[slot] = (r[ci][order] & (DQ - 1)).astype(np.float32)
        vv[slot] = v[ci][order].astype(np.float32)
        ii[slot] = idx_of_edge[ci][order].astype(np.int16)
        rr_t = np.ascontiguousarray(rr.reshape(tot_chunks, CHUNK).T)
        vv_t = np.ascontiguousarray(vv.reshape(tot_chunks, CHUNK).T)
        iw = np.ascontiguousarray(ii.reshape(tot_slots // 16, 16).T)
        iw = np.tile(iw, (8, 1))
        per_core.append((rr_t, vv_t, iw))
    return per_core, dict(tot_chunks=tot_chunks, tot_slots=tot_slots,
                          quad_chunk_off=quad_chunk_off, calls=calls,
                          chunk_par=chunk_par)


def _host_prep(x, rows, cols, vals, weight, bias):
    N, F = x.shape
    assert F == 64
    assert N % NC == 0
    shard = N // NC
    nquad = -(-shard // DQ)
    assert nquad % WINQ == 0
    nwin = nquad // WINQ
    vrows = nquad * DQ
    # feature tables are pair-packed fp16: row j = nodes (2j, 2j+1), 256B.
    # edges group by (source parity, pair-half); hp = pair rows per half.
    hp1 = ((-(-N // 2) + 1) // 2 + CHUNK - 1) // CHUNK * CHUNK   # spmm1
    hp2 = NC * vrows // 4                                        # spmm2
    assert hp1 < 32768 and hp2 < 32768
    assert (NC * vrows) % 4 == 0

    rows = np.asarray(rows).astype(np.int64)
    cols = np.asarray(cols).astype(np.int64)
    vals = np.asarray(vals, dtype=np.float32)
    x = np.asarray(x, dtype=np.float32)
    weight = np.asarray(weight, dtype=np.float32)
    bias = np.asarray(bias, dtype=np.float32)

    bounds = np.searchsorted(rows, np.arange(NC + 1) * shard)
    r_, c_, v_ = [], [], []
    for ci in range(NC):
        e0, e1 = bounds[ci], bounds[ci + 1]
        r_.append(rows[e0:e1] - ci * shard)
        c_.append(cols[e0:e1])
        v_.append(vals[e0:e1])

    # spmm1: gather pair row col>>1 from x_pad; group = half*2 + parity
    q1 = [(c >> 1) // hp1 * 2 + (c & 1) for c in c_]
    i1 = [(c >> 1) - ((c >> 1) // hp1) * hp1 for c in c_]
    # spmm2: gather from c_tbl (window AllGather -> window/rank/quad layout)
    # node row of node j: r=j//shard, lr=j-r*shard, tq=lr//DQ, w=tq//WINQ,
    #   row = w*(NC*WINQ*DQ) + r*(WINQ*DQ) + (tq%WINQ)*DQ + (lr%DQ)
    q2, i2 = [], []
    for c in c_:
        rr = c // shard
        lr = c - rr * shard
        tq = lr // DQ
        w = tq // WINQ
        trow = (w * (NC * WINQ * DQ) + rr * (WINQ * DQ)
                + (tq % WINQ) * DQ + (lr % DQ))
        pair = trow >> 1
        half = pair // hp2
        q2.append(half * 2 + (trow & 1))
        i2.append(pair - half * hp2)
    quad_dest = [r // DQ for r in r_]

    lay1_cores, lay1 = _edge_layout(q1, quad_dest, r_, c_, v_, i1, nquad)
    lay2_cores, lay2 = _edge_layout(q2, quad_dest, r_, c_, v_, i2, nquad)

    x_pad = np.zeros((2 * hp1, 2 * F), dtype=np.float16)
    x_pad.reshape(-1, F)[:N] = x.astype(np.float16)
    iota = np.tile(np.arange(DQ, dtype=np.float16), (128, 1))
    w1 = np.ascontiguousarray(weight[1].astype(np.float16))
    w2s = np.ascontiguousarray((2.0 * weight[2]).astype(np.float16))
    w0m2 = np.ascontiguousarray((weight[0] - weight[2]).astype(np.float16))
    biasT = np.ascontiguousarray(bias.reshape(F, 1))

    core_inputs = []
    for ci in range(NC):
        rr1, vv1, iw1 = lay1_cores[ci]
        rr2, vv2, iw2 = lay2_cores[ci]
        xq = np.zeros((F, vrows), dtype=np.float16)
        lo = ci * shard
        hi = min(lo + vrows, N)
        xq[:, :hi - lo] = x[lo:hi].T.astype(np.float16)
        core_inputs.append({
            "xg": x_pad, "xq": xq,
            "rr1": rr1, "vv1": vv1, "i1": iw1,
            "rr2": rr2, "vv2": vv2, "i2": iw2,
            "iota": iota, "w1": w1, "w2s": w2s, "w0m2": w0m2, "biasT": biasT,
        })

    meta = dict(N=N, F=F, shard=shard, nquad=nquad, nwin=nwin, vrows=vrows,
                hp1=hp1, hp2=hp2, lay1=lay1, lay2=lay2)
    return core_inputs, meta


def _build_program(meta):
    import concourse.bass as bass  # noqa
    import concourse.mybir as mybir
    import concourse.tile as tile
    from concourse import bacc

    F = meta["F"]
    nquad = meta["nquad"]
    nwin = meta["nwin"]
    vrows = meta["vrows"]
    hp1, hp2 = meta["hp1"], meta["hp2"]
    lay1, lay2 = meta["lay1"], meta["lay2"]
    f32, f16, i16 = mybir.dt.float32, mybir.dt.float16, mybir.dt.int16
    AOP = mybir.AluOpType
    ACTF = mybir.ActivationFunctionType

    nc = bacc.Bacc("TRN2", target_bir_lowering=False, debug=False,
                   num_devices=NC, num_swdge_queues=4)
    xg = nc.dram_tensor("xg", [2 * hp1, 2 * F], f16, kind="ExternalInput")
    xq = nc.dram_tensor("xq", [F, vrows], f16, kind="ExternalInput")
    edge_dram = {}
    for nm, lay in (("1", lay1), ("2", lay2)):
        edge_dram["rr" + nm] = nc.dram_tensor(
            "rr" + nm, [128, lay["tot_chunks"]], f32, kind="ExternalInput")
        edge_dram["vv" + nm] = nc.dram_tensor(
            "vv" + nm, [128, lay["tot_chunks"]], f32, kind="ExternalInput")
        edge_dram["i" + nm] = nc.dram_tensor(
            "i" + nm, [128, lay["tot_slots"] // 16], i16, kind="ExternalInput")
    iota = nc.dram_tensor("iota", [128, DQ], f16, kind="ExternalInput")
    w1 = nc.dram_tensor("w1", [F, F], f16, kind="ExternalInput")
    w2s = nc.dram_tensor("w2s", [F, F], f16, kind="ExternalInput")
    w0m2 = nc.dram_tensor("w0m2", [F, F], f16, kind="ExternalInput")
    biasT = nc.dram_tensor("biasT", [F, 1], f32, kind="ExternalInput")
    outT = nc.dram_tensor("outT", [F, vrows], f16, kind="ExternalOutput")
    c_shard = nc.dram_tensor("c_shard", [vrows, F], f16)
    c_tbl = nc.dram_tensor("c_tbl", [NC * vrows // 2, 2 * F], f16,
                           addr_space="Shared")

    max_qchunks = 0
    for lay in (lay1, lay2):
        qco = lay["quad_chunk_off"]
        max_qchunks = max(max_qchunks,
                          max(qco[t + 1] - qco[t] for t in range(nquad)))

    gq = [0]

    with tile.TileContext(nc) as tc:
        with tc.tile_pool(name="const", bufs=1) as constp, \
             tc.tile_pool(name="edges", bufs=8) as edgep, \
             tc.tile_pool(name="gbuf", bufs=6) as gp, \
             tc.tile_pool(name="mask", bufs=24) as mp, \
             tc.tile_pool(name="xqp", bufs=2) as xqp, \
             tc.tile_pool(name="acc", bufs=3) as accp, \
             tc.tile_pool(name="iops", bufs=1, space="PSUM") as iops, \
             tc.tile_pool(name="ps1", bufs=4, space="PSUM") as ps1, \
             tc.tile_pool(name="ps2", bufs=2, space="PSUM") as ps2:

            iota_t = constp.tile([128, DQ], f16)
            nc.sync.dma_start(out=iota_t[:], in_=iota[:])
            if os.environ.get("MASK_PSUM_IOTA", "0") == "1":
                # fp16 iota bit-packed in a fp32 PSUM tile: the PSUM operand
                # drops the mask tensor_scalar from 2-port modes to 2x_1p
                # (dedicated DVE port), so SWDGE descriptor generation on the
                # GpSimd Q7 is never locked out of the shared SBUF port pair.
                iota_ps = iops.tile([128, DQ // 2], f32)
                nc.scalar.activation(out=iota_ps[:],
                                     in_=iota_t[:].bitcast(f32),
                                     func=ACTF.Copy)
                mask_in0 = iota_ps[:].bitcast(f16)
            else:
                mask_in0 = iota_t[:]
            w1_t = constp.tile([F, F], f16, tag="w1")
            nc.sync.dma_start(out=w1_t[:], in_=w1[:])
            w2s_t = constp.tile([F, F], f16, tag="w2s")
            nc.sync.dma_start(out=w2s_t[:], in_=w2s[:])
            w0m2_t = constp.tile([F, F], f16, tag="w0m2")
            nc.sync.dma_start(out=w0m2_t[:], in_=w0m2[:])
            bias_t = constp.tile([F, 1], f32, tag="bias")
            nc.sync.dma_start(out=bias_t[:], in_=biasT[:])

            def spmm_quad(t, tbl, lay, nm, qsz, second):
                qco = lay["quad_chunk_off"]
                c0, c1 = qco[t], qco[t + 1]
                nch = c1 - c0
                rr_t = edgep.tile([128, max_qchunks], f32, tag="rr")
                nc.sync.dma_start(out=rr_t[:, :nch],
                                  in_=edge_dram["rr" + nm][:, c0:c1])
                vv_t = edgep.tile([128, max_qchunks], f32, tag="vv")
                nc.sync.dma_start(out=vv_t[:, :nch],
                                  in_=edge_dram["vv" + nm][:, c0:c1])
                ix_t = edgep.tile([128, max_qchunks * 8], i16, tag="ix")
                nc.sync.dma_start(out=ix_t[:, :nch * 8],
                                  in_=edge_dram["i" + nm][:, c0 * 8:c1 * 8])
                g16 = gp.tile([128, max_qchunks * 2 * F], f16, tag="g16")
                for (tt, q, k0, ncall) in lay["calls"]:
                    if tt != t:
                        continue
                    nidx = ncall * CHUNK
                    rel = k0 - c0
                    nc.gpsimd.dma_gather(
                        out_ap=g16[:, rel * 2 * F:(rel + ncall) * 2 * F]
                            .rearrange("p (c e) -> p c e", e=2 * F),
                        in_ap=tbl[(q >> 1) * qsz:, :],
                        idxs_ap=ix_t[:, rel * 8:rel * 8 + nidx // 16],
                        num_idxs=nidx, num_idxs_reg=nidx, elem_size=2 * F,
                        single_packet=os.environ.get("SP", "0") == "1",
                        queue_num=gq[0] % 4)
                    gq[0] += 1
                par = lay["chunk_par"]
                psum = ps1.tile([F, DQ], f32)
                for j in range(nch):
                    mask = mp.tile([128, DQ], f16)
                    nc.vector.tensor_scalar(
                        out=mask[:], in0=mask_in0,
                        scalar1=rr_t[:, j:j + 1], scalar2=vv_t[:, j:j + 1],
                        op0=AOP.is_equal, op1=AOP.mult)
                    lo = j * 2 * F + int(par[c0 + j]) * F
                    nc.tensor.matmul(out=psum[:],
                                     lhsT=g16[:, lo:lo + F],
                                     rhs=mask[:],
                                     start=(j == 0),
                                     stop=(j == nch - 1) and not second)
                xq_t = xqp.tile([F, DQ], f16, tag="xq")
                nc.sync.dma_start(out=xq_t[:], in_=xq[:, t * DQ:(t + 1) * DQ])
                if not second:
                    t1t = accp.tile([F, DQ], f16, tag="t1t")
                    nc.scalar.activation(out=t1t[:], in_=psum[:], func=ACTF.Copy)
                    ps = ps2.tile([128, (DQ // 128) * F], f32)
                    for k in range(DQ // 128):
                        nc.tensor.matmul(out=ps[:, k * F:(k + 1) * F],
                                         lhsT=t1t[:, k * 128:(k + 1) * 128],
                                         rhs=w2s_t[:], start=True, stop=False)
                        nc.tensor.matmul(out=ps[:, k * F:(k + 1) * F],
                                         lhsT=xq_t[:, k * 128:(k + 1) * 128],
                                         rhs=w1_t[:], start=False, stop=True)
                    c_sb = accp.tile([128, (DQ // 128) * F], f16, tag="csb")
                    nc.scalar.activation(out=c_sb[:], in_=ps[:], func=ACTF.Copy)
                    nc.sync.dma_start(
                        out=c_shard[t * DQ:(t + 1) * DQ, :]
                            .rearrange("(k p) e -> p k e", p=128),
                        in_=c_sb[:].rearrange("p (k e) -> p k e", e=F))
                else:
                    nc.tensor.matmul(out=psum[:], lhsT=w0m2_t[:], rhs=xq_t[:],
                                     start=False, stop=True)
                    o_sb = accp.tile([F, DQ], f16, tag="osb")
                    nc.scalar.activation(out=o_sb[:], in_=psum[:],
                                         func=ACTF.Identity, bias=bias_t[:])
                    nc.sync.dma_start(out=outT[:, t * DQ:(t + 1) * DQ],
                                      in_=o_sb[:])

            def emit_ag(w):
                nc.gpsimd.collective_compute(
                    "AllGather", mybir.AluOpType.bypass,
                    replica_groups=[list(range(NC))],
                    ins=[c_shard[w * WINQ * DQ:(w + 1) * WINQ * DQ, :]],
                    outs=[c_tbl[w * NC * WINQ * DQ // 2:
                                (w + 1) * NC * WINQ * DQ // 2, :]])

            for t in range(nquad):
                spmm_quad(t, xg, lay1, "1", hp1, second=False)
                if (t + 1) % WINQ == 0:
                    emit_ag((t + 1) // WINQ - 1)
            for t in range(nquad):
                spmm_quad(t, c_tbl, lay2, "2", hp2, second=True)

    nc.compile()
    return nc


def kernel(**inputs):
    global LAST_EXEC_NS
    core_inputs, meta = _host_prep(
        inputs["x"], inputs["rows"], inputs["cols"], inputs["vals"],
        inputs["weight"], inputs["bias"])
    nc = _build_program(meta)

    trace = os.environ.get("KERNEL_TRACE", "0") == "1"
    if trace:
        try:
            import sys, types  # noqa
            if "antenv.axon_hooks" not in sys.modules:
                import antenv
                from trn_agent_boot.trn_boot import _ntff_profile_via_ctypes
                mod = types.ModuleType("antenv.axon_hooks")
                hook = _ntff_profile_via_ctypes("/opt/axon/libaxon_pjrt.so")
                mod.get_axon_ntff_profile_hook = lambda: hook
                sys.modules["antenv.axon_hooks"] = mod
                antenv.axon_hooks = mod
        except Exception:
            trace = False

    from concourse.bass_utils import run_bass_kernel_spmd
    res = run_bass_kernel_spmd(nc, core_inputs, list(range(NC)), trace=trace)
    LAST_EXEC_NS = res.exec_time_ns

    N, F, shard = meta["N"], meta["F"], meta["shard"]
    out = np.empty((N, F), dtype=np.float32)
    for ci in range(NC):
        out[ci * shard:(ci + 1) * shard] = \
            res.results[ci]["outT"][:, :shard].T.astype(np.float32)
    return out


# revision 44
# speedup vs baseline: 1.3420x; 1.1277x over previous
"""ChebyConv (K=3) GNN kernel for 8 Trainium2 NeuronCores.

out = x@(W0-W2) + L@c + bias,  c = x@W1 + (L@x)@(2*W2)

Sharding: destination rows split across 8 cores. Edges (sorted by dest row)
are grouped per core by (dest-quad of DQ rows, source-quartile) and padded
to 128-edge chunks with a layout shared by all cores (SPMD single program).
Feature tables (x and the all-gathered c) are stored fp16 with 128-wide
rows (256B) so dma_gather lands rows directly in the fp16 lhsT layout.
Each SpMM chunk: DVE builds an fp16 selection mask [128 edges, DQ] from a
PSUM-resident iota (PSUM operand forces the 1-port DVE perf mode, which
does not block GpSimd SWDGE descriptor generation) -> PE fp16 matmul
accumulates out^T[64,DQ] in PSUM. Hop-1 results are AllGathered per
window of quads (pipelined under hop-1 compute) for the hop-2 gathers.
"""

import os
import numpy as np

CHUNK = 128          # edges per mask-matmul chunk (PE contraction dim)
DQ = 256             # dest rows per quad (mask free dim)
WINQ = 7             # quads per AllGather window
MAX_CALL_CHUNKS = 32  # 4096 indices per dma_gather call (single_packet=False)
NC = 8

LAST_EXEC_NS = None


def _edge_layout(q_of_edge, quad_of_edge, r, c, v, idx_of_edge, nquad):
    """Build the shared static slot layout for one spmm."""
    ngrp = nquad * 4
    counts = np.zeros((NC, ngrp), dtype=np.int64)
    keys = []
    orders = []
    for ci in range(NC):
        key = quad_of_edge[ci] * 4 + q_of_edge[ci]
        order = np.lexsort((c[ci], key))
        keys.append(key[order])
        orders.append(order)
        counts[ci] = np.bincount(key, minlength=ngrp)
    cg = np.maximum(1, -(-counts.max(axis=0) // CHUNK))
    grp_chunk_off = np.concatenate(([0], np.cumsum(cg)))
    tot_chunks = int(grp_chunk_off[-1])
    tot_slots = tot_chunks * CHUNK
    quad_chunk_off = [int(grp_chunk_off[t * 4]) for t in range(nquad)] + [tot_chunks]
    calls = []
    for t in range(nquad):
        for q in range(4):
            g = t * 4 + q
            c0, c1 = int(grp_chunk_off[g]), int(grp_chunk_off[g + 1])
            k = c0
            while k < c1:
                n = min(MAX_CALL_CHUNKS, c1 - k)
                calls.append((t, q, k, n))
                k += n

    # per-chunk parity: group g = quad*4 + (half*2 + parity); parity selects
    # which 64-column half of the gathered 256B pair-row holds the features
    chunk_par = np.zeros(tot_chunks, dtype=np.int64)
    for g in range(ngrp):
        chunk_par[grp_chunk_off[g]:grp_chunk_off[g + 1]] = g % 4 % 2

    per_core = []
    for ci in range(NC):
        order = orders[ci]
        key = keys[ci]
        cnt = counts[ci]
        rr = np.zeros(tot_slots, dtype=np.float32)
        vv = np.zeros(tot_slots, dtype=np.float32)
        ii = np.zeros(tot_slots, dtype=np.int16)
        within = np.arange(len(key)) - np.repeat(
            np.concatenate(([0], np.cumsum(cnt)))[:-1], cnt)
        slot = grp_chunk_off[key] * CHUNK + within
        rr[slot] = (r[ci][order] & (DQ - 1)).astype(np.float32)
        vv[slot] = v[ci][order].astype(np.float32)
        ii[slot] = idx_of_edge[ci][order].astype(np.int16)
        rr_t = np.ascontiguousarray(rr.reshape(tot_chunks, CHUNK).T)
        vv_t = np.ascontiguousarray(vv.reshape(tot_chunks, CHUNK).T)
        iw = np.ascontiguousarray(ii.reshape(tot_slots // 16, 16).T)
        iw = np.tile(iw, (8, 1))
        per_core.append((rr_t, vv_t, iw))
    return per_core, dict(tot_chunks=tot_chunks, tot_slots=tot_slots,
                          quad_chunk_off=quad_chunk_off, calls=calls,
                          chunk_par=chunk_par)


def _host_prep(x, rows, cols, vals, weight, bias):
    N, F = x.shape
    assert F == 64
    assert N % NC == 0
    shard = N // NC
    nquad = -(-shard // DQ)
    assert nquad % WINQ == 0
    nwin = nquad // WINQ
    vrows = nquad * DQ
    # feature tables are pair-packed fp16: row j = nodes (2j, 2j+1), 256B.
    # edges group by (source parity, pair-half); hp = pair rows per half.
    hp1 = ((-(-N // 2) + 1) // 2 + CHUNK - 1) // CHUNK * CHUNK   # spmm1
    hp2 = NC * vrows // 4                                        # spmm2
    assert hp1 < 32768 and hp2 < 32768
    assert (NC * vrows) % 4 == 0

    rows = np.asarray(rows).astype(np.int64)
    cols = np.asarray(cols).astype(np.int64)
    vals = np.asarray(vals, dtype=np.float32)
    x = np.asarray(x, dtype=np.float32)
    weight = np.asarray(weight, dtype=np.float32)
    bias = np.asarray(bias, dtype=np.float32)

    bounds = np.searchsorted(rows, np.arange(NC + 1) * shard)
    r_, c_, v_ = [], [], []
    for ci in range(NC):
        e0, e1 = bounds[ci], bounds[ci + 1]
        r_.append(rows[e0:e1] - ci * shard)
        c_.append(cols[e0:e1])
        v_.append(vals[e0:e1])

    # spmm1: gather pair row col>>1 from x_pad; group = half*2 + parity
    q1 = [(c >> 1) // hp1 * 2 + (c & 1) for c in c_]
    i1 = [(c >> 1) - ((c >> 1) // hp1) * hp1 for c in c_]
    # spmm2: gather from c_tbl (window AllGather -> window/rank/quad layout)
    # node row of node j: r=j//shard, lr=j-r*shard, tq=lr//DQ, w=tq//WINQ,
    #   row = w*(NC*WINQ*DQ) + r*(WINQ*DQ) + (tq%WINQ)*DQ + (lr%DQ)
    q2, i2 = [], []
    for c in c_:
        rr = c // shard
        lr = c - rr * shard
        tq = lr // DQ
        w = tq // WINQ
        trow = (w * (NC * WINQ * DQ) + rr * (WINQ * DQ)
                + (tq % WINQ) * DQ + (lr % DQ))
        pair = trow >> 1
        half = pair // hp2
        q2.append(half * 2 + (trow & 1))
        i2.append(pair - half * hp2)
    quad_dest = [r // DQ for r in r_]

    lay1_cores, lay1 = _edge_layout(q1, quad_dest, r_, c_, v_, i1, nquad)
    lay2_cores, lay2 = _edge_layout(q2, quad_dest, r_, c_, v_, i2, nquad)

    x_pad = np.zeros((2 * hp1, 2 * F), dtype=np.float16)
    x_pad.reshape(-1, F)[:N] = x.astype(np.float16)
    iota = np.tile(np.arange(DQ, dtype=np.float16), (128, 1))
    w1 = np.ascontiguousarray(weight[1].astype(np.float16))
    w2s = np.ascontiguousarray((2.0 * weight[2]).astype(np.float16))
    w0m2 = np.ascontiguousarray((weight[0] - weight[2]).astype(np.float16))
    biasT = np.ascontiguousarray(bias.reshape(F, 1))

    core_inputs = []
    for ci in range(NC):
        rr1, vv1, iw1 = lay1_cores[ci]
        rr2, vv2, iw2 = lay2_cores[ci]
        xq = np.zeros((F, vrows), dtype=np.float16)
        lo = ci * shard
        hi = min(lo + vrows, N)
        xq[:, :hi - lo] = x[lo:hi].T.astype(np.float16)
        core_inputs.append({
            "xg": x_pad, "xq": xq,
            "rr1": rr1, "vv1": vv1, "i1": iw1, "rrn1": -rr1,
            "rr2": rr2, "vv2": vv2, "i2": iw2, "rrn2": -rr2,
            "iota": iota, "w1": w1, "w2s": w2s, "w0m2": w0m2, "biasT": biasT,
        })

    meta = dict(N=N, F=F, shard=shard, nquad=nquad, nwin=nwin, vrows=vrows,
                hp1=hp1, hp2=hp2, lay1=lay1, lay2=lay2)
    return core_inputs, meta


def _build_program(meta):
    import concourse.bass as bass  # noqa
    import concourse.mybir as mybir
    import concourse.tile as tile
    from concourse import bacc

    F = meta["F"]
    nquad = meta["nquad"]
    nwin = meta["nwin"]
    vrows = meta["vrows"]
    hp1, hp2 = meta["hp1"], meta["hp2"]
    lay1, lay2 = meta["lay1"], meta["lay2"]
    f32, f16, i16 = mybir.dt.float32, mybir.dt.float16, mybir.dt.int16
    AOP = mybir.AluOpType
    ACTF = mybir.ActivationFunctionType

    nc = bacc.Bacc("TRN2", target_bir_lowering=False, debug=False,
                   num_devices=NC, num_swdge_queues=4)
    xg = nc.dram_tensor("xg", [2 * hp1, 2 * F], f16, kind="ExternalInput")
    xq = nc.dram_tensor("xq", [F, vrows], f16, kind="ExternalInput")
    edge_dram = {}
    for nm, lay in (("1", lay1), ("2", lay2)):
        edge_dram["rr" + nm] = nc.dram_tensor(
            "rr" + nm, [128, lay["tot_chunks"]], f32, kind="ExternalInput")
        edge_dram["rrn" + nm] = nc.dram_tensor(
            "rrn" + nm, [128, lay["tot_chunks"]], f32, kind="ExternalInput")
        edge_dram["vv" + nm] = nc.dram_tensor(
            "vv" + nm, [128, lay["tot_chunks"]], f32, kind="ExternalInput")
        edge_dram["i" + nm] = nc.dram_tensor(
            "i" + nm, [128, lay["tot_slots"] // 16], i16, kind="ExternalInput")
    iota = nc.dram_tensor("iota", [128, DQ], f16, kind="ExternalInput")
    w1 = nc.dram_tensor("w1", [F, F], f16, kind="ExternalInput")
    w2s = nc.dram_tensor("w2s", [F, F], f16, kind="ExternalInput")
    w0m2 = nc.dram_tensor("w0m2", [F, F], f16, kind="ExternalInput")
    biasT = nc.dram_tensor("biasT", [F, 1], f32, kind="ExternalInput")
    outT = nc.dram_tensor("outT", [F, vrows], f16, kind="ExternalOutput")
    c_shard = nc.dram_tensor("c_shard", [vrows, F], f16)
    c_tbl = nc.dram_tensor("c_tbl", [NC * vrows // 2, 2 * F], f16,
                           addr_space="Shared")

    max_qchunks = 0
    for lay in (lay1, lay2):
        qco = lay["quad_chunk_off"]
        max_qchunks = max(max_qchunks,
                          max(qco[t + 1] - qco[t] for t in range(nquad)))

    gq = [0]

    with tile.TileContext(nc) as tc:
        with tc.tile_pool(name="const", bufs=1) as constp, \
             tc.tile_pool(name="edges", bufs=8) as edgep, \
             tc.tile_pool(name="gbuf", bufs=6) as gp, \
             tc.tile_pool(name="mask", bufs=24) as mp, \
             tc.tile_pool(name="xqp", bufs=2) as xqp, \
             tc.tile_pool(name="actp", bufs=4) as actp, \
             tc.tile_pool(name="acc", bufs=3) as accp, \
             tc.tile_pool(name="iops", bufs=1, space="PSUM") as iops, \
             tc.tile_pool(name="ps1", bufs=4, space="PSUM") as ps1, \
             tc.tile_pool(name="ps2", bufs=2, space="PSUM") as ps2:

            iota_t = constp.tile([128, DQ], f16)
            nc.sync.dma_start(out=iota_t[:], in_=iota[:])
            if os.environ.get("MASK_PSUM_IOTA", "0") == "1":
                # fp16 iota bit-packed in a fp32 PSUM tile: the PSUM operand
                # drops the mask tensor_scalar from 2-port modes to 2x_1p
                # (dedicated DVE port), so SWDGE descriptor generation on the
                # GpSimd Q7 is never locked out of the shared SBUF port pair.
                iota_ps = iops.tile([128, DQ // 2], f32)
                nc.scalar.activation(out=iota_ps[:],
                                     in_=iota_t[:].bitcast(f32),
                                     func=ACTF.Copy)
                mask_in0 = iota_ps[:].bitcast(f16)
            else:
                mask_in0 = iota_t[:]
            w1_t = constp.tile([F, F], f16, tag="w1")
            nc.sync.dma_start(out=w1_t[:], in_=w1[:])
            w2s_t = constp.tile([F, F], f16, tag="w2s")
            nc.sync.dma_start(out=w2s_t[:], in_=w2s[:])
            w0m2_t = constp.tile([F, F], f16, tag="w0m2")
            nc.sync.dma_start(out=w0m2_t[:], in_=w0m2[:])
            bias_t = constp.tile([F, 1], f32, tag="bias")
            nc.sync.dma_start(out=bias_t[:], in_=biasT[:])

            def spmm_quad(t, tbl, lay, nm, qsz, second):
                qco = lay["quad_chunk_off"]
                c0, c1 = qco[t], qco[t + 1]
                nch = c1 - c0
                rr_t = edgep.tile([128, max_qchunks], f32, tag="rr")
                nc.sync.dma_start(out=rr_t[:, :nch],
                                  in_=edge_dram["rr" + nm][:, c0:c1])
                rrn_t = edgep.tile([128, max_qchunks], f32, tag="rrn")
                nc.sync.dma_start(out=rrn_t[:, :nch],
                                  in_=edge_dram["rrn" + nm][:, c0:c1])
                vv_t = edgep.tile([128, max_qchunks], f32, tag="vv")
                nc.sync.dma_start(out=vv_t[:, :nch],
                                  in_=edge_dram["vv" + nm][:, c0:c1])
                ix_t = edgep.tile([128, max_qchunks * 8], i16, tag="ix")
                nc.sync.dma_start(out=ix_t[:, :nch * 8],
                                  in_=edge_dram["i" + nm][:, c0 * 8:c1 * 8])
                g16 = gp.tile([128, max_qchunks * 2 * F], f16, tag="g16")
                for (tt, q, k0, ncall) in lay["calls"]:
                    if tt != t:
                        continue
                    nidx = ncall * CHUNK
                    rel = k0 - c0
                    nc.gpsimd.dma_gather(
                        out_ap=g16[:, rel * 2 * F:(rel + ncall) * 2 * F]
                            .rearrange("p (c e) -> p c e", e=2 * F),
                        in_ap=tbl[(q >> 1) * qsz:, :],
                        idxs_ap=ix_t[:, rel * 8:rel * 8 + nidx // 16],
                        num_idxs=nidx, num_idxs_reg=nidx, elem_size=2 * F,
                        single_packet=os.environ.get("SP", "0") == "1",
                        queue_num=gq[0] % 4)
                    gq[0] += 1
                par = lay["chunk_par"]
                psum = ps1.tile([F, DQ], f32)
                for j in range(nch):
                    mask = mp.tile([128, DQ], f16)
                    if (c0 + j) % 4 == 1:
                        # offload every 4th mask to the (otherwise idle)
                        # Scalar engine, which never contends for SBUF ports:
                        # t = |iota - rr|; mask = relu(vv - 2t)
                        t_t = actp.tile([128, DQ], f16)
                        nc.scalar.activation(out=t_t[:], in_=iota_t[:],
                                             func=ACTF.Abs,
                                             bias=rrn_t[:, j:j + 1])
                        nc.scalar.activation(out=mask[:], in_=t_t[:],
                                             func=ACTF.Relu, scale=-2.0,
                                             bias=vv_t[:, j:j + 1])
                    else:
                        nc.vector.tensor_scalar(
                            out=mask[:], in0=mask_in0,
                            scalar1=rr_t[:, j:j + 1], scalar2=vv_t[:, j:j + 1],
                            op0=AOP.is_equal, op1=AOP.mult)
                    lo = j * 2 * F + int(par[c0 + j]) * F
                    nc.tensor.matmul(out=psum[:],
                                     lhsT=g16[:, lo:lo + F],
                                     rhs=mask[:],
                                     start=(j == 0),
                                     stop=(j == nch - 1) and not second)
                xq_t = xqp.tile([F, DQ], f16, tag="xq")
                nc.sync.dma_start(out=xq_t[:], in_=xq[:, t * DQ:(t + 1) * DQ])
                if not second:
                    t1t = accp.tile([F, DQ], f16, tag="t1t")
                    nc.scalar.activation(out=t1t[:], in_=psum[:], func=ACTF.Copy)
                    ps = ps2.tile([128, (DQ // 128) * F], f32)
                    for k in range(DQ // 128):
                        nc.tensor.matmul(out=ps[:, k * F:(k + 1) * F],
                                         lhsT=t1t[:, k * 128:(k + 1) * 128],
                                         rhs=w2s_t[:], start=True, stop=False)
                        nc.tensor.matmul(out=ps[:, k * F:(k + 1) * F],
                                         lhsT=xq_t[:, k * 128:(k + 1) * 128],
                                         rhs=w1_t[:], start=False, stop=True)
                    c_sb = accp.tile([128, (DQ // 128) * F], f16, tag="csb")
                    nc.scalar.activation(out=c_sb[:], in_=ps[:], func=ACTF.Copy)
                    nc.sync.dma_start(
                        out=c_shard[t * DQ:(t + 1) * DQ, :]
                            .rearrange("(k p) e -> p k e", p=128),
                        in_=c_sb[:].rearrange("p (k e) -> p k e", e=F))
                else:
                    nc.tensor.matmul(out=psum[:], lhsT=w0m2_t[:], rhs=xq_t[:],
                                     start=False, stop=True)
                    o_sb = accp.tile([F, DQ], f16, tag="osb")
                    nc.scalar.activation(out=o_sb[:], in_=psum[:],
                                         func=ACTF.Identity, bias=bias_t[:])
                    nc.sync.dma_start(out=outT[:, t * DQ:(t + 1) * DQ],
                                      in_=o_sb[:])

            def emit_ag(w):
                nc.gpsimd.collective_compute(
                    "AllGather", mybir.AluOpType.bypass,
                    replica_groups=[list(range(NC))],
                    ins=[c_shard[w * WINQ * DQ:(w + 1) * WINQ * DQ, :]],
                    outs=[c_tbl[w * NC * WINQ * DQ // 2:
                                (w + 1) * NC * WINQ * DQ // 2, :]])

            for t in range(nquad):
                spmm_quad(t, xg, lay1, "1", hp1, second=False)
                if (t + 1) % WINQ == 0:
                    emit_ag((t + 1) // WINQ - 1)
            for t in range(nquad):
                spmm_quad(t, c_tbl, lay2, "2", hp2, second=True)

    nc.compile()
    return nc


def kernel(**inputs):
    global LAST_EXEC_NS
    core_inputs, meta = _host_prep(
        inputs["x"], inputs["rows"], inputs["cols"], inputs["vals"],
        inputs["weight"], inputs["bias"])
    nc = _build_program(meta)

    trace = os.environ.get("KERNEL_TRACE", "0") == "1"
    if trace:
        try:
            import sys, types  # noqa
            if "antenv.axon_hooks" not in sys.modules:
                import antenv
                from trn_agent_boot.trn_boot import _ntff_profile_via_ctypes
                mod = types.ModuleType("antenv.axon_hooks")
                hook = _ntff_profile_via_ctypes("/opt/axon/libaxon_pjrt.so")
                mod.get_axon_ntff_profile_hook = lambda: hook
                sys.modules["antenv.axon_hooks"] = mod
                antenv.axon_hooks = mod
        except Exception:
            trace = False

    from concourse.bass_utils import run_bass_kernel_spmd
    res = run_bass_kernel_spmd(nc, core_inputs, list(range(NC)), trace=trace)
    LAST_EXEC_NS = res.exec_time_ns

    N, F, shard = meta["N"], meta["F"], meta["shard"]
    out = np.empty((N, F), dtype=np.float32)
    for ci in range(NC):
        out[ci * shard:(ci + 1) * shard] = \
            res.results[ci]["outT"][:, :shard].T.astype(np.float32)
    return out


# revision 45
# speedup vs baseline: 1.3566x; 1.0108x over previous
"""ChebyConv (K=3) GNN kernel for 8 Trainium2 NeuronCores.

out = x@(W0-W2) + L@c + bias,  c = x@W1 + (L@x)@(2*W2)

Sharding: destination rows split across 8 cores. Edges (sorted by dest row)
are grouped per core by (dest-quad of DQ rows, source-quartile) and padded
to 128-edge chunks with a layout shared by all cores (SPMD single program).
Feature tables (x and the all-gathered c) are stored fp16 with 128-wide
rows (256B) so dma_gather lands rows directly in the fp16 lhsT layout.
Each SpMM chunk: DVE builds an fp16 selection mask [128 edges, DQ] from a
PSUM-resident iota (PSUM operand forces the 1-port DVE perf mode, which
does not block GpSimd SWDGE descriptor generation) -> PE fp16 matmul
accumulates out^T[64,DQ] in PSUM. Hop-1 results are AllGathered per
window of quads (pipelined under hop-1 compute) for the hop-2 gathers.
"""

import os
import numpy as np

CHUNK = 128          # edges per mask-matmul chunk (PE contraction dim)
DQ = 256             # dest rows per quad (mask free dim)
WINQ = 7             # quads per AllGather window
MAX_CALL_CHUNKS = 32  # 4096 indices per dma_gather call (single_packet=False)
NC = 8

LAST_EXEC_NS = None


def _edge_layout(q_of_edge, quad_of_edge, r, c, v, idx_of_edge, nquad):
    """Build the shared static slot layout for one spmm."""
    ngrp = nquad * 4
    counts = np.zeros((NC, ngrp), dtype=np.int64)
    keys = []
    orders = []
    for ci in range(NC):
        key = quad_of_edge[ci] * 4 + q_of_edge[ci]
        order = np.lexsort((c[ci], key))
        keys.append(key[order])
        orders.append(order)
        counts[ci] = np.bincount(key, minlength=ngrp)
    cg = np.maximum(1, -(-counts.max(axis=0) // CHUNK))
    grp_chunk_off = np.concatenate(([0], np.cumsum(cg)))
    tot_chunks = int(grp_chunk_off[-1])
    tot_slots = tot_chunks * CHUNK
    quad_chunk_off = [int(grp_chunk_off[t * 4]) for t in range(nquad)] + [tot_chunks]
    calls = []
    for t in range(nquad):
        for q in range(4):
            g = t * 4 + q
            c0, c1 = int(grp_chunk_off[g]), int(grp_chunk_off[g + 1])
            k = c0
            while k < c1:
                n = min(MAX_CALL_CHUNKS, c1 - k)
                calls.append((t, q, k, n))
                k += n

    # per-chunk parity: group g = quad*4 + (half*2 + parity); parity selects
    # which 64-column half of the gathered 256B pair-row holds the features
    chunk_par = np.zeros(tot_chunks, dtype=np.int64)
    for g in range(ngrp):
        chunk_par[grp_chunk_off[g]:grp_chunk_off[g + 1]] = g % 4 % 2

    per_core = []
    for ci in range(NC):
        order = orders[ci]
        key = keys[ci]
        cnt = counts[ci]
        rr = np.zeros(tot_slots, dtype=np.float32)
        vv = np.zeros(tot_slots, dtype=np.float32)
        ii = np.zeros(tot_slots, dtype=np.int16)
        within = np.arange(len(key)) - np.repeat(
            np.concatenate(([0], np.cumsum(cnt)))[:-1], cnt)
        slot = grp_chunk_off[key] * CHUNK + within
        rr[slot] = (r[ci][order] & (DQ - 1)).astype(np.float32)
        vv[slot] = v[ci][order].astype(np.float32)
        ii[slot] = idx_of_edge[ci][order].astype(np.int16)
        rr_t = np.ascontiguousarray(rr.reshape(tot_chunks, CHUNK).T)
        vv_t = np.ascontiguousarray(vv.reshape(tot_chunks, CHUNK).T)
        iw = np.ascontiguousarray(ii.reshape(tot_slots // 16, 16).T)
        iw = np.tile(iw, (8, 1))
        per_core.append((rr_t, vv_t, iw))
    return per_core, dict(tot_chunks=tot_chunks, tot_slots=tot_slots,
                          quad_chunk_off=quad_chunk_off, calls=calls,
                          chunk_par=chunk_par)


def _host_prep(x, rows, cols, vals, weight, bias):
    N, F = x.shape
    assert F == 64
    assert N % NC == 0
    shard = N // NC
    nquad = -(-shard // DQ)
    assert nquad % WINQ == 0
    nwin = nquad // WINQ
    vrows = nquad * DQ
    # feature tables are pair-packed fp16: row j = nodes (2j, 2j+1), 256B.
    # edges group by (source parity, pair-half); hp = pair rows per half.
    hp1 = ((-(-N // 2) + 1) // 2 + CHUNK - 1) // CHUNK * CHUNK   # spmm1
    hp2 = NC * vrows // 4                                        # spmm2
    assert hp1 < 32768 and hp2 < 32768
    assert (NC * vrows) % 4 == 0

    rows = np.asarray(rows).astype(np.int64)
    cols = np.asarray(cols).astype(np.int64)
    vals = np.asarray(vals, dtype=np.float32)
    x = np.asarray(x, dtype=np.float32)
    weight = np.asarray(weight, dtype=np.float32)
    bias = np.asarray(bias, dtype=np.float32)

    bounds = np.searchsorted(rows, np.arange(NC + 1) * shard)
    r_, c_, v_ = [], [], []
    for ci in range(NC):
        e0, e1 = bounds[ci], bounds[ci + 1]
        r_.append(rows[e0:e1] - ci * shard)
        c_.append(cols[e0:e1])
        v_.append(vals[e0:e1])

    # spmm1: gather pair row col>>1 from x_pad; group = half*2 + parity
    q1 = [(c >> 1) // hp1 * 2 + (c & 1) for c in c_]
    i1 = [(c >> 1) - ((c >> 1) // hp1) * hp1 for c in c_]
    # spmm2: gather from c_tbl (window AllGather -> window/rank/quad layout)
    # node row of node j: r=j//shard, lr=j-r*shard, tq=lr//DQ, w=tq//WINQ,
    #   row = w*(NC*WINQ*DQ) + r*(WINQ*DQ) + (tq%WINQ)*DQ + (lr%DQ)
    q2, i2 = [], []
    for c in c_:
        rr = c // shard
        lr = c - rr * shard
        tq = lr // DQ
        w = tq // WINQ
        trow = (w * (NC * WINQ * DQ) + rr * (WINQ * DQ)
                + (tq % WINQ) * DQ + (lr % DQ))
        pair = trow >> 1
        half = pair // hp2
        q2.append(half * 2 + (trow & 1))
        i2.append(pair - half * hp2)
    quad_dest = [r // DQ for r in r_]

    lay1_cores, lay1 = _edge_layout(q1, quad_dest, r_, c_, v_, i1, nquad)
    lay2_cores, lay2 = _edge_layout(q2, quad_dest, r_, c_, v_, i2, nquad)

    x_pad = np.zeros((2 * hp1, 2 * F), dtype=np.float16)
    x_pad.reshape(-1, F)[:N] = x.astype(np.float16)
    iota = np.tile(np.arange(DQ, dtype=np.float16), (128, 1))
    w1 = np.ascontiguousarray(weight[1].astype(np.float16))
    w2s = np.ascontiguousarray((2.0 * weight[2]).astype(np.float16))
    w0m2 = np.ascontiguousarray((weight[0] - weight[2]).astype(np.float16))
    biasT = np.ascontiguousarray(bias.reshape(F, 1))

    core_inputs = []
    for ci in range(NC):
        rr1, vv1, iw1 = lay1_cores[ci]
        rr2, vv2, iw2 = lay2_cores[ci]
        xq = np.zeros((F, vrows), dtype=np.float16)
        lo = ci * shard
        hi = min(lo + vrows, N)
        xq[:, :hi - lo] = x[lo:hi].T.astype(np.float16)
        core_inputs.append({
            "xg": x_pad, "xq": xq,
            "rr1": rr1, "vv1": vv1, "i1": iw1, "rrn1": -rr1,
            "rr2": rr2, "vv2": vv2, "i2": iw2, "rrn2": -rr2,
            "iota": iota, "w1": w1, "w2s": w2s, "w0m2": w0m2, "biasT": biasT,
        })

    meta = dict(N=N, F=F, shard=shard, nquad=nquad, nwin=nwin, vrows=vrows,
                hp1=hp1, hp2=hp2, lay1=lay1, lay2=lay2)
    return core_inputs, meta


def _build_program(meta):
    import concourse.bass as bass  # noqa
    import concourse.mybir as mybir
    import concourse.tile as tile
    from concourse import bacc

    F = meta["F"]
    nquad = meta["nquad"]
    nwin = meta["nwin"]
    vrows = meta["vrows"]
    hp1, hp2 = meta["hp1"], meta["hp2"]
    lay1, lay2 = meta["lay1"], meta["lay2"]
    f32, f16, i16 = mybir.dt.float32, mybir.dt.float16, mybir.dt.int16
    AOP = mybir.AluOpType
    ACTF = mybir.ActivationFunctionType

    nc = bacc.Bacc("TRN2", target_bir_lowering=False, debug=False,
                   num_devices=NC, num_swdge_queues=4)
    xg = nc.dram_tensor("xg", [2 * hp1, 2 * F], f16, kind="ExternalInput")
    xq = nc.dram_tensor("xq", [F, vrows], f16, kind="ExternalInput")
    edge_dram = {}
    for nm, lay in (("1", lay1), ("2", lay2)):
        edge_dram["rr" + nm] = nc.dram_tensor(
            "rr" + nm, [128, lay["tot_chunks"]], f32, kind="ExternalInput")
        edge_dram["rrn" + nm] = nc.dram_tensor(
            "rrn" + nm, [128, lay["tot_chunks"]], f32, kind="ExternalInput")
        edge_dram["vv" + nm] = nc.dram_tensor(
            "vv" + nm, [128, lay["tot_chunks"]], f32, kind="ExternalInput")
        edge_dram["i" + nm] = nc.dram_tensor(
            "i" + nm, [128, lay["tot_slots"] // 16], i16, kind="ExternalInput")
    iota = nc.dram_tensor("iota", [128, DQ], f16, kind="ExternalInput")
    w1 = nc.dram_tensor("w1", [F, F], f16, kind="ExternalInput")
    w2s = nc.dram_tensor("w2s", [F, F], f16, kind="ExternalInput")
    w0m2 = nc.dram_tensor("w0m2", [F, F], f16, kind="ExternalInput")
    biasT = nc.dram_tensor("biasT", [F, 1], f32, kind="ExternalInput")
    outT = nc.dram_tensor("outT", [F, vrows], f16, kind="ExternalOutput")
    c_shard = nc.dram_tensor("c_shard", [vrows, F], f16)
    c_tbl = nc.dram_tensor("c_tbl", [NC * vrows // 2, 2 * F], f16,
                           addr_space="Shared")

    max_qchunks = 0
    for lay in (lay1, lay2):
        qco = lay["quad_chunk_off"]
        max_qchunks = max(max_qchunks,
                          max(qco[t + 1] - qco[t] for t in range(nquad)))

    gq = [0]

    with tile.TileContext(nc) as tc:
        with tc.tile_pool(name="const", bufs=1) as constp, \
             tc.tile_pool(name="edges", bufs=8) as edgep, \
             tc.tile_pool(name="gbuf", bufs=6) as gp, \
             tc.tile_pool(name="mask", bufs=24) as mp, \
             tc.tile_pool(name="xqp", bufs=2) as xqp, \
             tc.tile_pool(name="actp", bufs=4) as actp, \
             tc.tile_pool(name="acc", bufs=3) as accp, \
             tc.tile_pool(name="iops", bufs=1, space="PSUM") as iops, \
             tc.tile_pool(name="ps1", bufs=4, space="PSUM") as ps1, \
             tc.tile_pool(name="ps2", bufs=2, space="PSUM") as ps2:

            iota_t = constp.tile([128, DQ], f16)
            nc.sync.dma_start(out=iota_t[:], in_=iota[:])
            if os.environ.get("MASK_PSUM_IOTA", "0") == "1":
                # fp16 iota bit-packed in a fp32 PSUM tile: the PSUM operand
                # drops the mask tensor_scalar from 2-port modes to 2x_1p
                # (dedicated DVE port), so SWDGE descriptor generation on the
                # GpSimd Q7 is never locked out of the shared SBUF port pair.
                iota_ps = iops.tile([128, DQ // 2], f32)
                nc.scalar.activation(out=iota_ps[:],
                                     in_=iota_t[:].bitcast(f32),
                                     func=ACTF.Copy)
                mask_in0 = iota_ps[:].bitcast(f16)
            else:
                mask_in0 = iota_t[:]
            w1_t = constp.tile([F, F], f16, tag="w1")
            nc.sync.dma_start(out=w1_t[:], in_=w1[:])
            w2s_t = constp.tile([F, F], f16, tag="w2s")
            nc.sync.dma_start(out=w2s_t[:], in_=w2s[:])
            w0m2_t = constp.tile([F, F], f16, tag="w0m2")
            nc.sync.dma_start(out=w0m2_t[:], in_=w0m2[:])
            bias_t = constp.tile([F, 1], f32, tag="bias")
            nc.sync.dma_start(out=bias_t[:], in_=biasT[:])

            def spmm_quad(t, tbl, lay, nm, qsz, second):
                qco = lay["quad_chunk_off"]
                c0, c1 = qco[t], qco[t + 1]
                nch = c1 - c0
                rr_t = edgep.tile([128, max_qchunks], f32, tag="rr")
                nc.sync.dma_start(out=rr_t[:, :nch],
                                  in_=edge_dram["rr" + nm][:, c0:c1])
                rrn_t = edgep.tile([128, max_qchunks], f32, tag="rrn")
                nc.sync.dma_start(out=rrn_t[:, :nch],
                                  in_=edge_dram["rrn" + nm][:, c0:c1])
                vv_t = edgep.tile([128, max_qchunks], f32, tag="vv")
                nc.sync.dma_start(out=vv_t[:, :nch],
                                  in_=edge_dram["vv" + nm][:, c0:c1])
                ix_t = edgep.tile([128, max_qchunks * 8], i16, tag="ix")
                nc.sync.dma_start(out=ix_t[:, :nch * 8],
                                  in_=edge_dram["i" + nm][:, c0 * 8:c1 * 8])
                g16 = gp.tile([128, max_qchunks * 2 * F], f16, tag="g16")
                for (tt, q, k0, ncall) in lay["calls"]:
                    if tt != t:
                        continue
                    nidx = ncall * CHUNK
                    rel = k0 - c0
                    nc.gpsimd.dma_gather(
                        out_ap=g16[:, rel * 2 * F:(rel + ncall) * 2 * F]
                            .rearrange("p (c e) -> p c e", e=2 * F),
                        in_ap=tbl[(q >> 1) * qsz:, :],
                        idxs_ap=ix_t[:, rel * 8:rel * 8 + nidx // 16],
                        num_idxs=nidx, num_idxs_reg=nidx, elem_size=2 * F,
                        single_packet=os.environ.get("SP", "0") == "1",
                        queue_num=gq[0] % 4)
                    gq[0] += 1
                par = lay["chunk_par"]
                psum = ps1.tile([F, DQ], f32)
                for j in range(nch):
                    mask = mp.tile([128, DQ], f16)
                    if (c0 + j) % 9 < 4:
                        # offload every 4th mask to the (otherwise idle)
                        # Scalar engine, which never contends for SBUF ports:
                        # t = |iota - rr|; mask = relu(vv - 2t)
                        t_t = actp.tile([128, DQ], f16)
                        nc.scalar.activation(out=t_t[:], in_=iota_t[:],
                                             func=ACTF.Abs,
                                             bias=rrn_t[:, j:j + 1])
                        nc.scalar.activation(out=mask[:], in_=t_t[:],
                                             func=ACTF.Relu, scale=-2.0,
                                             bias=vv_t[:, j:j + 1])
                    else:
                        nc.vector.tensor_scalar(
                            out=mask[:], in0=mask_in0,
                            scalar1=rr_t[:, j:j + 1], scalar2=vv_t[:, j:j + 1],
                            op0=AOP.is_equal, op1=AOP.mult)
                    lo = j * 2 * F + int(par[c0 + j]) * F
                    nc.tensor.matmul(out=psum[:],
                                     lhsT=g16[:, lo:lo + F],
                                     rhs=mask[:],
                                     start=(j == 0),
                                     stop=(j == nch - 1) and not second)
                xq_t = xqp.tile([F, DQ], f16, tag="xq")
                nc.sync.dma_start(out=xq_t[:], in_=xq[:, t * DQ:(t + 1) * DQ])
                if not second:
                    t1t = accp.tile([F, DQ], f16, tag="t1t")
                    nc.scalar.activation(out=t1t[:], in_=psum[:], func=ACTF.Copy)
                    ps = ps2.tile([128, (DQ // 128) * F], f32)
                    for k in range(DQ // 128):
                        nc.tensor.matmul(out=ps[:, k * F:(k + 1) * F],
                                         lhsT=t1t[:, k * 128:(k + 1) * 128],
                                         rhs=w2s_t[:], start=True, stop=False)
                        nc.tensor.matmul(out=ps[:, k * F:(k + 1) * F],
                                         lhsT=xq_t[:, k * 128:(k + 1) * 128],
                                         rhs=w1_t[:], start=False, stop=True)
                    c_sb = accp.tile([128, (DQ // 128) * F], f16, tag="csb")
                    nc.scalar.activation(out=c_sb[:], in_=ps[:], func=ACTF.Copy)
                    nc.sync.dma_start(
                        out=c_shard[t * DQ:(t + 1) * DQ, :]
                            .rearrange("(k p) e -> p k e", p=128),
                        in_=c_sb[:].rearrange("p (k e) -> p k e", e=F))
                else:
                    nc.tensor.matmul(out=psum[:], lhsT=w0m2_t[:], rhs=xq_t[:],
                                     start=False, stop=True)
                    o_sb = accp.tile([F, DQ], f16, tag="osb")
                    nc.scalar.activation(out=o_sb[:], in_=psum[:],
                                         func=ACTF.Identity, bias=bias_t[:])
                    nc.sync.dma_start(out=outT[:, t * DQ:(t + 1) * DQ],
                                      in_=o_sb[:])

            def emit_ag(w):
                nc.gpsimd.collective_compute(
                    "AllGather", mybir.AluOpType.bypass,
                    replica_groups=[list(range(NC))],
                    ins=[c_shard[w * WINQ * DQ:(w + 1) * WINQ * DQ, :]],
                    outs=[c_tbl[w * NC * WINQ * DQ // 2:
                                (w + 1) * NC * WINQ * DQ // 2, :]])

            for t in range(nquad):
                spmm_quad(t, xg, lay1, "1", hp1, second=False)
                if (t + 1) % WINQ == 0:
                    emit_ag((t + 1) // WINQ - 1)
            for t in range(nquad):
                spmm_quad(t, c_tbl, lay2, "2", hp2, second=True)

    nc.compile()
    return nc


def kernel(**inputs):
    global LAST_EXEC_NS
    core_inputs, meta = _host_prep(
        inputs["x"], inputs["rows"], inputs["cols"], inputs["vals"],
        inputs["weight"], inputs["bias"])
    nc = _build_program(meta)

    trace = os.environ.get("KERNEL_TRACE", "0") == "1"
    if trace:
        try:
            import sys, types  # noqa
            if "antenv.axon_hooks" not in sys.modules:
                import antenv
                from trn_agent_boot.trn_boot import _ntff_profile_via_ctypes
                mod = types.ModuleType("antenv.axon_hooks")
                hook = _ntff_profile_via_ctypes("/opt/axon/libaxon_pjrt.so")
                mod.get_axon_ntff_profile_hook = lambda: hook
                sys.modules["antenv.axon_hooks"] = mod
                antenv.axon_hooks = mod
        except Exception:
            trace = False

    from concourse.bass_utils import run_bass_kernel_spmd
    res = run_bass_kernel_spmd(nc, core_inputs, list(range(NC)), trace=trace)
    LAST_EXEC_NS = res.exec_time_ns

    N, F, shard = meta["N"], meta["F"], meta["shard"]
    out = np.empty((N, F), dtype=np.float32)
    for ci in range(NC):
        out[ci * shard:(ci + 1) * shard] = \
            res.results[ci]["outT"][:, :shard].T.astype(np.float32)
    return out
